# revision 13
# baseline (speedup 1.0000x reference)
"""Local (sliding-window causal) attention kernel for Trainium2, 8 NeuronCores.

Reference computation (per batch b, head h):
  q = x @ Wq + bq ; k = x @ Wk + bk ; v = x @ Wv + bv   (split into 16 heads of 64)
  S = q k^T / 8, masked to the causal band  i-255 <= j <= i
  out = softmax(S) @ v

Sharding: B=2, H=16 -> 32 (b,h) units; each of 8 cores owns 2 heads x 2 batches
(= a 128-wide column slice of the QKV projections and of the output). Inputs are
replicated (hidden_states as a pre-transposed bf16 x^T) and weights are column-
sliced per core, so no collectives are needed.

Device-side scheme per core (all matmuls in bf16, accumulating in fp32 PSUM):
  1. Q^T, K^T = W^T @ x^T   -> [128 (2 heads*64), 4096] layout (dh on partitions)
  2. V       = x @ Wv       -> [tokens, 128] layout (tokens on partitions),
               stored per 128-token block with a ones-column appended: V' = [V | 1]
  3. Per (b, h, key-block kb of 128 keys): the only queries attending these keys
     are the 384 starting at kb*128, so one matmul
        S^T[kb] = K^T[kb-block].T @ Q^T[:, window]   ([128 keys, <=384 queries])
     + additive band mask + exp (no max-subtraction needed: |scores| < ~4)
     gives P~^T. Then for each 128-query block qb in the window:
        O~[qb] (+)= P~^T[:, qb].T @ V'[kb]           ([128 q, 65]; col 64 = row sums)
     accumulated in PSUM over the <=3 contributing key blocks; finally
     out[qb] = O~[:, :64] * (1 / O~[:, 64]).
bv is folded in on the host: softmax rows sum to 1, so P @ (1 bv^T) = bv.
"""

import os
import sys

import numpy as np

try:
    import concourse.bass as bass  # noqa: F401
except ImportError:
    sys.path.insert(0, "/opt/trn_rl_repo")

import concourse.bass as bass
import concourse.tile as tile
from concourse import bacc, mybir
from concourse.bass import ts
from concourse.bass_utils import run_bass_kernel_spmd

import ml_dtypes

P = 128
B, L, D = 2, 2048, 1024
NT = B * L            # 4096 tokens
KSUB = D // P         # 8 contraction subtiles
CHUNK = 512           # projection chunk (tokens)
NCH = NT // CHUNK     # 8
NLB = NT // P         # 32 token blocks
NKB = L // P          # 16 key blocks per batch
QW = 384              # query window per key block
DH = 64               # head dim
NCORES = 8
HEADS_PER_CORE = 2

F32 = mybir.dt.float32
BF16 = mybir.dt.bfloat16

VARIANT = "full"  # bisect hook: full | proj | projv | noatt... (see build_program)


def build_program():
    nc = bacc.Bacc("TRN2", target_bir_lowering=False, debug=False,
                   num_devices=NCORES)

    xt_d = nc.dram_tensor("xt", [P, KSUB, NT], BF16, kind="ExternalInput").ap()
    wq_d = nc.dram_tensor("wq", [P, KSUB, P], BF16, kind="ExternalInput").ap()
    wk_d = nc.dram_tensor("wk", [P, KSUB, P], BF16, kind="ExternalInput").ap()
    wv_d = nc.dram_tensor("wv", [P, KSUB, P], BF16, kind="ExternalInput").ap()
    bq_d = nc.dram_tensor("bq", [P, 1], F32, kind="ExternalInput").ap()
    bk_d = nc.dram_tensor("bk", [P, 1], F32, kind="ExternalInput").ap()
    mask_d = nc.dram_tensor("mask", [P, QW], F32, kind="ExternalInput").ap()
    out_d = nc.dram_tensor("out", [B, L, P], F32, kind="ExternalOutput").ap()

    with tile.TileContext(nc) as tc:
        with (
            tc.tile_pool(name="const", bufs=1) as const,
            tc.tile_pool(name="xtp", bufs=1) as xtp,
            tc.tile_pool(name="qkv", bufs=1) as qkv,
        ):
            mask_sb = const.tile([P, QW], F32)
            nc.sync.dma_start(mask_sb[:], mask_d)
            wq_sb = const.tile([P, KSUB, P], BF16)
            nc.sync.dma_start(wq_sb[:], wq_d)
            wk_sb = const.tile([P, KSUB, P], BF16)
            nc.sync.dma_start(wk_sb[:], wk_d)
            wv_sb = const.tile([P, KSUB, P], BF16)
            nc.sync.dma_start(wv_sb[:], wv_d)
            bq_sb = const.tile([P, 1], F32)
            nc.sync.dma_start(bq_sb[:], bq_d)
            bk_sb = const.tile([P, 1], F32)
            nc.sync.dma_start(bk_sb[:], bk_d)

            qt_sb = qkv.tile([P, NT], BF16, tag="qt")   # Q^T (2 heads on partitions)
            kt_sb = qkv.tile([P, NT], BF16, tag="kt")   # K^T
            v_sb = qkv.tile([P, HEADS_PER_CORE, NLB, DH + 1], BF16, tag="v")
            nc.vector.memset(v_sb[:, :, :, DH:DH + 1], 1.0)

            xts = []
            for c in range(NCH):
                t = xtp.tile([P, KSUB, CHUNK], BF16, tag=f"xt{c}")
                nc.sync.dma_start(t[:], xt_d[:, :, ts(c, CHUNK)])
                xts.append(t)

            do_p1 = VARIANT in ("full", "p1", "p12", "p13")
            do_p2 = VARIANT in ("full", "p12", "p2")
            do_p3 = VARIANT in ("full", "p13")
            if not do_p3:
                dummy = qkv.tile([P, DH], F32, tag="dummy")
                nc.vector.memset(dummy[:], 0.0)
                for b in range(B):
                    for qb in range(NKB):
                        for h in range(HEADS_PER_CORE):
                            nc.sync.dma_start(
                                out_d[b, qb * P:(qb + 1) * P,
                                      h * DH:(h + 1) * DH], dummy[:])

            # ---- Phase 1: Q^T / K^T projections ----
            with tc.tile_pool(name="pjps", bufs=3, space="PSUM") as pj_ps:
                for c in range(NCH if do_p1 else 0):
                    for w_sb, b_sb, dst in ((wq_sb, bq_sb, qt_sb),
                                            (wk_sb, bk_sb, kt_sb)):
                        ps = pj_ps.tile([P, CHUNK], F32, tag="pj")
                        for k in range(KSUB):
                            nc.tensor.matmul(ps[:], lhsT=w_sb[:, k, :],
                                             rhs=xts[c][:, k, :],
                                             start=(k == 0), stop=(k == KSUB - 1))
                        nc.vector.tensor_scalar_add(dst[:, ts(c, CHUNK)], ps[:],
                                                    b_sb[:, 0:1])

            # ---- Phase 2: V projection (token-major layout) ----
            with tc.tile_pool(name="vps", bufs=3, space="PSUM") as v_ps:
                for lb in range(NLB if do_p2 else 0):
                    c, off = lb // 4, (lb % 4) * P
                    ps = v_ps.tile([P, P], F32, tag="v")
                    for k in range(KSUB):
                        nc.tensor.matmul(ps[:], lhsT=xts[c][:, k, off:off + P],
                                         rhs=wv_sb[:, k, :],
                                         start=(k == 0), stop=(k == KSUB - 1))
                    for h in range(HEADS_PER_CORE):
                        nc.vector.tensor_copy(v_sb[:, h, lb, 0:DH],
                                              ps[:, h * DH:(h + 1) * DH])

            # ---- Phase 3: banded attention ----
            with (
                tc.tile_pool(name="stps", bufs=2, space="PSUM") as st_ps,
                tc.tile_pool(name="ops", bufs=6, space="PSUM") as o_ps,
                tc.tile_pool(name="att", bufs=6) as att,
                tc.tile_pool(name="ptp", bufs=14) as ptp,
                tc.tile_pool(name="osb", bufs=6) as osb,
            ):
                # Four streams (2 batches x 2 heads) interleaved per kb step
                # so PE always has independent matmuls while DVE/ACT run the
                # mask+exp of other streams. The two heads of a batch share
                # one PSUM o-bank ([*, 0:65] / [*, 65:130]) to fit 8 banks.
                o_tiles = {0: {}, 1: {}}
                o_outs = {}
                for kb in range(NKB if do_p3 else 0):
                    qw = min(QW, L - kb * P)
                    for b in range(B):
                        t0 = b * L
                        k0 = t0 + kb * P
                        for h in range(HEADS_PER_CORE):
                            hs = h * DH
                            ps_st = st_ps.tile([P, QW], F32, tag="st")
                            nc.tensor.matmul(ps_st[:, :qw],
                                             lhsT=kt_sb[hs:hs + DH, k0:k0 + P],
                                             rhs=qt_sb[hs:hs + DH, k0:k0 + qw],
                                             start=True, stop=True)
                            st_sb = att.tile([P, QW], F32, tag="st_sb")
                            nc.vector.tensor_add(st_sb[:, :qw], ps_st[:, :qw],
                                                 mask_sb[:, :qw])
                            pt_sb = ptp.tile([P, QW], BF16, tag="pt")
                            nc.scalar.activation(
                                pt_sb[:, :qw], st_sb[:, :qw],
                                mybir.ActivationFunctionType.Exp, scale=0.125)
                            for qb in range(kb, min(kb + 3, NKB)):
                                qoff = (qb - kb) * P
                                first = (kb == max(qb - 2, 0))
                                last = (qb == kb)
                                if first and h == 0:
                                    o_tiles[b][qb] = o_ps.tile(
                                        [P, 2 * (DH + 1)], F32, tag="o",
                                        name=f"o_{b}_{qb}")
                                osl = o_tiles[b][qb][:, h * (DH + 1):
                                                     (h + 1) * (DH + 1)]
                                # start=True clears has_written for the WHOLE
                                # bank, so only h0 may issue it; h1's first
                                # matmul lands on freshly cleared bits and
                                # overwrites, later ones accumulate.
                                nc.tensor.matmul(
                                    osl,
                                    lhsT=pt_sb[:, qoff:qoff + P],
                                    rhs=v_sb[:, h, b * NKB + kb, :],
                                    start=first and h == 0, stop=last,
                                    skip_group_check=True)
                                if last:
                                    ot = o_tiles[b][qb]
                                    if h == 1:
                                        o_tiles[b].pop(qb)
                                    c0 = h * (DH + 1)
                                    r = osb.tile([P, 1], F32, tag="r")
                                    nc.vector.reciprocal(
                                        r[:], ot[:, c0 + DH:c0 + DH + 1])
                                    if h == 0:
                                        o_outs[(b, qb)] = osb.tile(
                                            [P, 2 * DH], F32, tag="oo",
                                            name=f"oo_{b}_{qb}")
                                    o_out = o_outs[(b, qb)]
                                    nc.vector.tensor_scalar_mul(
                                        o_out[:, hs:hs + DH],
                                        ot[:, c0:c0 + DH], r[:])
                                    if h == 1:
                                        nc.sync.dma_start(
                                            out_d[b, qb * P:(qb + 1) * P, :],
                                            o_outs.pop((b, qb))[:])
    nc.finalize()
    return nc


_NC = None


def _get_nc():
    global _NC
    if _NC is None:
        _NC = build_program()
    return _NC


def _band_mask():
    pk = np.arange(P)[:, None]
    fq = np.arange(QW)[None, :]
    valid = (fq >= pk) & (fq - pk <= 255)
    return np.where(valid, 0.0, -30000.0).astype(np.float32)


def _prepare_in_maps(inputs):
    hs = np.asarray(inputs["hidden_states"], np.float32)
    Wq = np.asarray(inputs["Wq"], np.float32)
    Wk = np.asarray(inputs["Wk"], np.float32)
    Wv = np.asarray(inputs["Wv"], np.float32)
    bq = np.asarray(inputs["bq"], np.float32)
    bk = np.asarray(inputs["bk"], np.float32)

    x_flat = hs.reshape(NT, D)
    # xt[p, k, t] = x_flat[t, k*128+p]
    xt = np.ascontiguousarray(
        x_flat.T.reshape(KSUB, P, NT).transpose(1, 0, 2)
    ).astype(ml_dtypes.bfloat16)
    mask = _band_mask()

    def wslice(W, c):
        # [P, KSUB, 128]: w[p, k, m] = W[k*128+p, c*128+m]
        return np.ascontiguousarray(
            W[:, c * P:(c + 1) * P].reshape(KSUB, P, P).transpose(1, 0, 2)
        ).astype(ml_dtypes.bfloat16)

    in_maps = []
    for c in range(NCORES):
        in_maps.append({
            "xt": xt,
            "wq": wslice(Wq, c),
            "wk": wslice(Wk, c),
            "wv": wslice(Wv, c),
            "bq": np.ascontiguousarray(bq[c * P:(c + 1) * P].reshape(P, 1)),
            "bk": np.ascontiguousarray(bk[c * P:(c + 1) * P].reshape(P, 1)),
            "mask": mask,
        })
    return in_maps


def run(inputs, trace=False, **kwargs):
    nc = _get_nc()
    in_maps = _prepare_in_maps(inputs)
    res = run_bass_kernel_spmd(nc, in_maps, core_ids=list(range(NCORES)),
                               trace=trace, **kwargs)
    bv = np.asarray(inputs["bv"], np.float32)
    full = np.concatenate([res.results[c]["out"] for c in range(NCORES)],
                          axis=2)
    full = full + bv[None, None, :]
    return full.astype(np.float32), res


def kernel(**inputs):
    out, _ = run(inputs, trace=False)
    return out


# revision 14
# speedup vs baseline: 1.2096x; 1.2096x over previous
"""Local (sliding-window causal) attention kernel for Trainium2, 8 NeuronCores.

Reference computation (per batch b, head h):
  q = x @ Wq + bq ; k = x @ Wk + bk ; v = x @ Wv + bv   (split into 16 heads of 64)
  S = q k^T / 8, masked to the causal band  i-255 <= j <= i
  out = softmax(S) @ v

Sharding: B=2, H=16 -> 32 (b,h) units; each of 8 cores owns 2 heads x 2 batches
(= a 128-wide column slice of the QKV projections and of the output). Inputs are
replicated (hidden_states as a pre-transposed bf16 x^T) and weights are column-
sliced per core, so no collectives are needed.

Device-side scheme per core (all matmuls in bf16, accumulating in fp32 PSUM):
  1. Q^T, K^T = W^T @ x^T   -> [128 (2 heads*64), 4096] layout (dh on partitions)
  2. V       = x @ Wv       -> [tokens, 128] layout (tokens on partitions),
               stored per 128-token block with a ones-column appended: V' = [V | 1]
  3. Per (b, h, key-block kb of 128 keys): the only queries attending these keys
     are the 384 starting at kb*128, so one matmul
        S^T[kb] = K^T[kb-block].T @ Q^T[:, window]   ([128 keys, <=384 queries])
     + additive band mask + exp (no max-subtraction needed: |scores| < ~4)
     gives P~^T. Then for each 128-query block qb in the window:
        O~[qb] (+)= P~^T[:, qb].T @ V'[kb]           ([128 q, 65]; col 64 = row sums)
     accumulated in PSUM over the <=3 contributing key blocks; finally
     out[qb] = O~[:, :64] * (1 / O~[:, 64]).
bv is folded in on the host: softmax rows sum to 1, so P @ (1 bv^T) = bv.
"""

import os
import sys

import numpy as np

try:
    import concourse.bass as bass  # noqa: F401
except ImportError:
    sys.path.insert(0, "/opt/trn_rl_repo")

import concourse.bass as bass
import concourse.tile as tile
from concourse import bacc, mybir
from concourse.bass import ts
from concourse.bass_utils import run_bass_kernel_spmd

import ml_dtypes

P = 128
B, L, D = 2, 2048, 1024
NT = B * L            # 4096 tokens
KSUB = D // P         # 8 contraction subtiles
CHUNK = 512           # projection chunk (tokens)
NCH = NT // CHUNK     # 8
NLB = NT // P         # 32 token blocks
NKB = L // P          # 16 key blocks per batch
QW = 384              # query window per key block
DH = 64               # head dim
NCORES = 8
HEADS_PER_CORE = 2

F32 = mybir.dt.float32
BF16 = mybir.dt.bfloat16

VARIANT = "full"  # bisect hook: full | proj | projv | noatt... (see build_program)


def build_program():
    nc = bacc.Bacc("TRN2", target_bir_lowering=False, debug=False,
                   num_devices=NCORES)

    xt_d = nc.dram_tensor("xt", [P, KSUB, NT], BF16, kind="ExternalInput").ap()
    wq_d = nc.dram_tensor("wq", [P, KSUB, P], BF16, kind="ExternalInput").ap()
    wk_d = nc.dram_tensor("wk", [P, KSUB, P], BF16, kind="ExternalInput").ap()
    wv_d = nc.dram_tensor("wv", [P, KSUB, P], BF16, kind="ExternalInput").ap()
    bq_d = nc.dram_tensor("bq", [P, 1], F32, kind="ExternalInput").ap()
    bk_d = nc.dram_tensor("bk", [P, 1], F32, kind="ExternalInput").ap()
    mask_d = nc.dram_tensor("mask", [P, QW], F32, kind="ExternalInput").ap()
    out_d = nc.dram_tensor("out", [B, L, P], F32, kind="ExternalOutput").ap()

    with tile.TileContext(nc) as tc:
        with (
            tc.tile_pool(name="const", bufs=1) as const,
            tc.tile_pool(name="xtp", bufs=1) as xtp,
            tc.tile_pool(name="qkv", bufs=1) as qkv,
        ):
            mask_sb = const.tile([P, QW], F32)
            nc.sync.dma_start(mask_sb[:], mask_d)
            wq_sb = const.tile([P, KSUB, P], BF16)
            nc.sync.dma_start(wq_sb[:], wq_d)
            wk_sb = const.tile([P, KSUB, P], BF16)
            nc.sync.dma_start(wk_sb[:], wk_d)
            wv_sb = const.tile([P, KSUB, P], BF16)
            nc.sync.dma_start(wv_sb[:], wv_d)
            bq_sb = const.tile([P, 1], F32)
            nc.sync.dma_start(bq_sb[:], bq_d)
            bk_sb = const.tile([P, 1], F32)
            nc.sync.dma_start(bk_sb[:], bk_d)

            qt_sb = qkv.tile([P, NT], BF16, tag="qt")   # Q^T (2 heads on partitions)
            kt_sb = qkv.tile([P, NT], BF16, tag="kt")   # K^T
            v_sb = qkv.tile([P, HEADS_PER_CORE, NLB, DH + 1], BF16, tag="v")
            nc.vector.memset(v_sb[:, :, :, DH:DH + 1], 1.0)

            xts = []
            for c in range(NCH):
                t = xtp.tile([P, KSUB, CHUNK], BF16, tag=f"xt{c}")
                nc.sync.dma_start(t[:], xt_d[:, :, ts(c, CHUNK)])
                xts.append(t)

            do_p1 = VARIANT in ("full", "p1", "p12", "p13")
            do_p2 = VARIANT in ("full", "p12", "p2")
            do_p3 = VARIANT in ("full", "p13")
            if not do_p3:
                dummy = qkv.tile([P, DH], F32, tag="dummy")
                nc.vector.memset(dummy[:], 0.0)
                for b in range(B):
                    for qb in range(NKB):
                        for h in range(HEADS_PER_CORE):
                            nc.sync.dma_start(
                                out_d[b, qb * P:(qb + 1) * P,
                                      h * DH:(h + 1) * DH], dummy[:])

            # ---- Fused per-batch pipeline: projections + attention ----
            # Attention key-blocks issue as soon as their 384-token QT/KT
            # window and V' blocks exist, so ACT/DVE softmax work overlaps
            # the projection matmuls instead of running after them.
            with (
                tc.tile_pool(name="pjps", bufs=2, space="PSUM") as pj_ps,
                tc.tile_pool(name="vps", bufs=1, space="PSUM") as v_ps,
                tc.tile_pool(name="stps", bufs=2, space="PSUM") as st_ps,
                tc.tile_pool(name="ops", bufs=3, space="PSUM") as o_ps,
                tc.tile_pool(name="att", bufs=6) as att,
                tc.tile_pool(name="ptp", bufs=8) as ptp,
                tc.tile_pool(name="osb", bufs=6) as osb,
            ):
                def attend(b, kb, o_tiles, o_outs):
                    t0 = b * L
                    k0 = t0 + kb * P
                    qw = min(QW, L - kb * P)
                    for h in range(HEADS_PER_CORE):
                        hs = h * DH
                        ps_st = st_ps.tile([P, QW], F32, tag="st", name="ps_st")
                        nc.tensor.matmul(ps_st[:, :qw],
                                         lhsT=kt_sb[hs:hs + DH, k0:k0 + P],
                                         rhs=qt_sb[hs:hs + DH, k0:k0 + qw],
                                         start=True, stop=True)
                        st_sb = att.tile([P, QW], F32, tag="st_sb",
                                         name="st_sb")
                        nc.vector.tensor_add(st_sb[:, :qw], ps_st[:, :qw],
                                             mask_sb[:, :qw])
                        pt_sb = ptp.tile([P, QW], BF16, tag="pt", name="pt_sb")
                        nc.scalar.activation(
                            pt_sb[:, :qw], st_sb[:, :qw],
                            mybir.ActivationFunctionType.Exp, scale=0.125)
                        for qb in range(kb, min(kb + 3, NKB)):
                            qoff = (qb - kb) * P
                            first = (kb == max(qb - 2, 0))
                            last = (qb == kb)
                            if first and h == 0:
                                o_tiles[qb] = o_ps.tile(
                                    [P, 2 * (DH + 1)], F32, tag="o",
                                    name=f"o_{b}_{qb}")
                            osl = o_tiles[qb][:, h * (DH + 1):
                                              (h + 1) * (DH + 1)]
                            # start=True clears has_written for the WHOLE
                            # bank, so only h0 may issue it; h1's first
                            # matmul lands on freshly cleared bits and
                            # overwrites, later ones accumulate.
                            nc.tensor.matmul(
                                osl,
                                lhsT=pt_sb[:, qoff:qoff + P],
                                rhs=v_sb[:, h, b * NKB + kb, :],
                                start=first and h == 0, stop=last,
                                skip_group_check=True)
                            if last:
                                ot = o_tiles[qb]
                                if h == 1:
                                    o_tiles.pop(qb)
                                c0 = h * (DH + 1)
                                r = osb.tile([P, 1], F32, tag="r", name="r")
                                nc.vector.reciprocal(
                                    r[:], ot[:, c0 + DH:c0 + DH + 1])
                                if h == 0:
                                    o_outs[qb] = osb.tile(
                                        [P, 2 * DH], F32, tag="oo",
                                        name=f"oo_{b}_{qb}")
                                o_out = o_outs[qb]
                                nc.vector.tensor_scalar_mul(
                                    o_out[:, hs:hs + DH],
                                    ot[:, c0:c0 + DH], r[:])
                                if h == 1:
                                    nc.sync.dma_start(
                                        out_d[b, qb * P:(qb + 1) * P, :],
                                        o_outs.pop(qb)[:])

                # kbs whose QT/KT window completes with local chunk cc
                ready = {0: [0, 1], 1: [2, 3, 4, 5], 2: [6, 7, 8, 9],
                         3: [10, 11, 12, 13]}
                for b in range(B if (do_p1 and do_p2 and do_p3) else 0):
                    o_tiles, o_outs = {}, {}
                    for cc in range(4):
                        c = b * 4 + cc
                        for w_sb, b_sb, dst in ((wq_sb, bq_sb, qt_sb),
                                                (wk_sb, bk_sb, kt_sb)):
                            ps = pj_ps.tile([P, CHUNK], F32, tag="pj",
                                            name="pj")
                            for k in range(KSUB):
                                nc.tensor.matmul(ps[:], lhsT=w_sb[:, k, :],
                                                 rhs=xts[c][:, k, :],
                                                 start=(k == 0),
                                                 stop=(k == KSUB - 1))
                            nc.vector.tensor_scalar_add(dst[:, ts(c, CHUNK)],
                                                        ps[:], b_sb[:, 0:1])
                        for lo in range(4):
                            lb = c * 4 + lo
                            ps = v_ps.tile([P, P], F32, tag="v", name="vps")
                            for k in range(KSUB):
                                nc.tensor.matmul(
                                    ps[:], lhsT=xts[c][:, k, ts(lo, P)],
                                    rhs=wv_sb[:, k, :],
                                    start=(k == 0), stop=(k == KSUB - 1))
                            for h in range(HEADS_PER_CORE):
                                nc.vector.tensor_copy(
                                    v_sb[:, h, lb, 0:DH],
                                    ps[:, h * DH:(h + 1) * DH])
                        for kb in ready[cc]:
                            attend(b, kb, o_tiles, o_outs)
                    for kb in (14, 15):
                        attend(b, kb, o_tiles, o_outs)
    nc.finalize()
    return nc


_NC = None


def _get_nc():
    global _NC
    if _NC is None:
        _NC = build_program()
    return _NC


def _band_mask():
    pk = np.arange(P)[:, None]
    fq = np.arange(QW)[None, :]
    valid = (fq >= pk) & (fq - pk <= 255)
    return np.where(valid, 0.0, -30000.0).astype(np.float32)


def _prepare_in_maps(inputs):
    hs = np.asarray(inputs["hidden_states"], np.float32)
    Wq = np.asarray(inputs["Wq"], np.float32)
    Wk = np.asarray(inputs["Wk"], np.float32)
    Wv = np.asarray(inputs["Wv"], np.float32)
    bq = np.asarray(inputs["bq"], np.float32)
    bk = np.asarray(inputs["bk"], np.float32)

    x_flat = hs.reshape(NT, D)
    # xt[p, k, t] = x_flat[t, k*128+p]
    xt = np.ascontiguousarray(
        x_flat.T.reshape(KSUB, P, NT).transpose(1, 0, 2)
    ).astype(ml_dtypes.bfloat16)
    mask = _band_mask()

    def wslice(W, c):
        # [P, KSUB, 128]: w[p, k, m] = W[k*128+p, c*128+m]
        return np.ascontiguousarray(
            W[:, c * P:(c + 1) * P].reshape(KSUB, P, P).transpose(1, 0, 2)
        ).astype(ml_dtypes.bfloat16)

    in_maps = []
    for c in range(NCORES):
        in_maps.append({
            "xt": xt,
            "wq": wslice(Wq, c),
            "wk": wslice(Wk, c),
            "wv": wslice(Wv, c),
            "bq": np.ascontiguousarray(bq[c * P:(c + 1) * P].reshape(P, 1)),
            "bk": np.ascontiguousarray(bk[c * P:(c + 1) * P].reshape(P, 1)),
            "mask": mask,
        })
    return in_maps


def run(inputs, trace=False, **kwargs):
    nc = _get_nc()
    in_maps = _prepare_in_maps(inputs)
    res = run_bass_kernel_spmd(nc, in_maps, core_ids=list(range(NCORES)),
                               trace=trace, **kwargs)
    bv = np.asarray(inputs["bv"], np.float32)
    full = np.concatenate([res.results[c]["out"] for c in range(NCORES)],
                          axis=2)
    full = full + bv[None, None, :]
    return full.astype(np.float32), res


def kernel(**inputs):
    out, _ = run(inputs, trace=False)
    return out
